# revision 1
# baseline (speedup 1.0000x reference)
"""Causal multi-head attention (B=2, N=2048, C=768, H=12, D=64) on 8 trn2 cores.

Sharding: 8 cores = 2 batches x 4 head-blocks (3 heads each). Each core
computes q/k/v projections for its 3 heads, causal flash-attention, and a
partial output projection (its 192 columns of Wo). Host sums the 4 partials
per batch (the "all-reduce") and adds the bias during the gather.

v2 (bf16): all matmul operands bf16 (fp32 psum accumulation), which keeps
1 cycle/row on PE at any free size, doubles DVE throughput on 16-bit ops and
halves DMA traffic. Structural changes vs v1:
  - QK proj packed into 3 m-blocks (m2 = [Q2|K2]); K2 moved to partition
    base 0 via a tiny SBUF->SBUF DMA so the S^T matmul operands share a base.
  - Output proj contraction 128+64 (heads 0,1 packed into o_pk[128,:] via a
    [64,512] SBUF->SBUF DMA after normalize) -> 2 matmuls per ob instead of 3.
  - S^T matmuls and exp causally clipped per group (free = [glo:512)).
  - st/pv emission pipelined per head so PE never waits on the exp chase.
  - Z rows gathered with one DMA per q-block; normalize chain in fp32.
"""

import numpy as np
import ml_dtypes

B, N, C = 2, 2048, 768
H, D = 12, 64
HL = 3            # heads per core
HD = HL * D       # 192
KC = C // 128     # 6 contraction chunks
NT = N // 512     # 4 query blocks
SCALE = D ** -0.5  # 0.125

_CACHED_NC = None


def _build_nc():
    global _CACHED_NC
    if _CACHED_NC is not None:
        return _CACHED_NC
    import concourse.mybir as mybir
    import concourse.tile as tile
    from concourse import bacc

    f32 = mybir.dt.float32
    bf16 = mybir.dt.bfloat16
    Exp = mybir.ActivationFunctionType.Exp
    is_ge = mybir.AluOpType.is_ge

    nc = bacc.Bacc("TRN2", target_bir_lowering=False, debug=False,
                   enable_asserts=False, num_devices=8)
    xt_d = nc.dram_tensor("xt", [C, N], bf16, kind="ExternalInput").ap()
    wqk_d = nc.dram_tensor("wqk", [C, 384], bf16, kind="ExternalInput").ap()
    wv_d = nc.dram_tensor("wv", [C, 192], bf16, kind="ExternalInput").ap()
    wo_d = nc.dram_tensor("wo", [HD, C], bf16, kind="ExternalInput").ap()
    yt_d = nc.dram_tensor("yt", [C, N], bf16, kind="ExternalOutput").ap()

    # per-head (m-chunk, base-partition) in qkt_sb. K2 lives in k2t_sb.
    QLOC = [(0, 0), (0, 64), (2, 0)]
    KLOC = [(1, 0), (1, 64), None]

    with tile.TileContext(nc) as tc:
        with (
            tc.tile_pool(name="wpool", bufs=1) as wpool,
            tc.tile_pool(name="qkpool", bufs=1) as qkpool,
            tc.tile_pool(name="vpool", bufs=1) as vpool,
            tc.tile_pool(name="opool", bufs=2) as opool,
            tc.tile_pool(name="ppsum", bufs=3, space="PSUM") as ppsum,
            tc.tile_pool(name="stpsum", bufs=2, space="PSUM") as stpsum,
            tc.tile_pool(name="otpsum", bufs=1, space="PSUM") as otpsum,
        ):
            wqk_sb = wpool.tile([128, KC, 384], bf16)
            wv_sb = wpool.tile([128, KC, 192], bf16)
            wopk_sb = wpool.tile([128, C], bf16)
            wo2_sb = wpool.tile([64, C], bf16)
            qkt_sb = qkpool.tile([128, 3, N], bf16)
            k2t_sb = qkpool.tile([64, N], bf16)
            v_sb = vpool.tile([128, HL * 16, 65], bf16)
            ones_sb = wpool.tile([128, 1], bf16)

            xpool_cm = tc.tile_pool(name="xpool", bufs=1)
            xpool = xpool_cm.__enter__()
            xt_sb = xpool.tile([128, KC, N], bf16)

            # ---- loads: consumption-ordered, spread across 3 hwdge queues ----
            xt_r = xt_d.rearrange("(c p) n -> p c n", p=128)
            wqk_r = wqk_d.rearrange("(c p) m -> p c m", p=128)
            wv_r = wv_d.rearrange("(c p) m -> p c m", p=128)
            engs = [nc.sync, nc.scalar]
            for m in range(3):
                ms = slice(m * 128, (m + 1) * 128)
                engs[m % 2].dma_start(wqk_sb[:, :, ms], wqk_r[:, :, ms])
                cs = slice(m * 2, m * 2 + 2)
                engs[(m + 1) % 2].dma_start(xt_sb[:, cs, 0:512],
                                            xt_r[:, cs, 0:512])
            nc.gpsimd.dma_start(wv_sb[:], wv_r[:])
            nc.vector.memset(ones_sb[:], 1.0)
            nc.vector.tensor_copy(v_sb[:, :, 64:65],
                                  ones_sb[:].to_broadcast([128, HL * 16, 1]))
            for t in range(1, NT):
                for cp in range(3):
                    cs = slice(cp * 2, cp * 2 + 2)
                    sl = slice(t * 512, (t + 1) * 512)
                    engs[(t + cp) % 2].dma_start(xt_sb[:, cs, sl],
                                                 xt_r[:, cs, sl])
            nc.scalar.dma_start(wopk_sb[:], wo_d[0:128, :])
            nc.gpsimd.dma_start(wo2_sb[:], wo_d[128:HD, :])

            # ---- projections, as independent psum-group units (PE fillers) ----
            def proj_units(t):
                ts_ = slice(t * 512, (t + 1) * 512)
                units = []

                def qk_unit(m):
                    def emit():
                        ps = ppsum.tile([128, 512], f32, tag="proj")
                        for c in range(KC):
                            nc.tensor.matmul(ps[:], wqk_sb[:, c, m * 128:(m + 1) * 128],
                                             xt_sb[:, c, ts_],
                                             start=(c == 0), stop=(c == KC - 1))
                        nc.vector.tensor_copy(qkt_sb[:, m, ts_], ps[:])
                        if m == 2:
                            nc.sync.dma_start(k2t_sb[:, ts_],
                                              qkt_sb[64:128, 2, ts_])
                    return emit

                def v_unit(q):
                    def emit():
                        tok = t * 512 + q * 128
                        i = t * 4 + q
                        ps = ppsum.tile([128, 512], f32, tag="proj")
                        for c in range(KC):
                            nc.tensor.matmul(ps[:, 0:192], xt_sb[:, c, tok:tok + 128],
                                             wv_sb[:, c, :],
                                             start=(c == 0), stop=(c == KC - 1))
                        vsrc = ps[:, 0:HD].rearrange("p (h d) -> p h d", h=HL)
                        nc.vector.tensor_copy(v_sb[:, i::16, 0:64], vsrc)
                    return emit

                for m in (0, 1):
                    units.append(qk_unit(m))
                for q in range(4):
                    units.append(v_unit(q))
                units.append(qk_unit(2))
                return units

            def emit_proj(t):
                for u in proj_units(t):
                    u()

            def yt_units(j, o_pk, o_j2, ytpool):
                qs = slice(j * 512, (j + 1) * 512)
                yt_r = yt_d.rearrange("(c p) n -> p c n", p=128)
                yt_sb = ytpool.tile([128, KC, 512], bf16, tag="yt")
                units = []

                def ob_unit(ob):
                    def emit():
                        ps = ppsum.tile([128, 512], f32, tag="proj")
                        nc.tensor.matmul(ps[:], wopk_sb[:, ob * 128:(ob + 1) * 128],
                                         o_pk[:], start=True, stop=False)
                        nc.tensor.matmul(ps[:], wo2_sb[:, ob * 128:(ob + 1) * 128],
                                         o_j2[:], start=False, stop=True)
                        if ob % 2 == 0:
                            nc.scalar.copy(yt_sb[:, ob, :], ps[:])
                        else:
                            nc.vector.tensor_copy(yt_sb[:, ob, :], ps[:])
                        if ob == KC - 1:
                            nc.sync.dma_start(yt_r[:, :, qs], yt_sb[:])
                    return emit

                return [ob_unit(ob) for ob in range(KC)]

            def emit_attention(j, ptpool, smpool, fillers):
                qs0 = j * 512
                o_f = opool.tile([65, HL, 512], f32, tag="of")
                o_pk = opool.tile([128, 512], bf16, tag="opk")
                o_m1 = opool.tile([64, 512], bf16, tag="om1")
                o_j2 = opool.tile([64, 512], bf16, tag="oj2")
                zpl = opool.tile([1, HL, 512], f32, tag="zpl")
                nkb = 4 * (j + 1)
                ngr = 2 * (j + 1)
                nslots = HL * nkb
                cadence = max(1, nslots // (len(fillers) + 1))
                slot = [0]

                for h in range(HL):
                    qm, qp = QLOC[h]
                    ot = otpsum.tile([65, 512], f32, tag="ot")
                    pts = []

                    def st_group(g):
                        glo = 128 * (2 * g - 4 * j) if 2 * g >= 4 * j else 0
                        st = stpsum.tile([128, 2, 512], f32, tag="st")
                        pt = ptpool.tile([128, 2, 512], bf16, tag="pt")
                        pts.append(pt)
                        for li in range(2):
                            kb = 2 * g + li
                            if h < 2:
                                km, kp = KLOC[h]
                                lhsk = qkt_sb[kp:kp + 64, km,
                                              kb * 128:(kb + 1) * 128]
                            else:
                                lhsk = k2t_sb[:, kb * 128:(kb + 1) * 128]
                            nc.tensor.matmul(
                                st[:, li, glo:512], lhsk,
                                qkt_sb[qp:qp + 64, qm, qs0 + glo:qs0 + 512],
                                start=True, stop=True)
                        nc.scalar.activation(pt[:, :, glo:512], st[:, :, glo:512],
                                             Exp, scale=SCALE)
                        for li in range(2):
                            kb = 2 * g + li
                            if kb >= 4 * j:  # diagonal: mask the 128-wide band
                                di = kb - 4 * j
                                blk = pt[:, li, 128 * di:128 * (di + 1)]
                                nc.gpsimd.affine_select(
                                    blk, blk, pattern=[[1, 128]], compare_op=is_ge,
                                    fill=0.0, base=0, channel_multiplier=-1)

                    def pv_group(g):
                        pt = pts[g]
                        for li in range(2):
                            kb = 2 * g + li
                            lo = 128 * (kb - 4 * j) if kb >= 4 * j else 0
                            nc.tensor.matmul(ot[:, lo:512], v_sb[:, h * 16 + kb, :],
                                             pt[:, li, lo:512],
                                             start=(kb == 0), stop=(kb == nkb - 1))
                            slot[0] += 1
                            if fillers and slot[0] % cadence == 0:
                                fillers.popleft()()

                    # software-pipelined: st group g+1 overlaps pv group g
                    st_group(0)
                    for g in range(1, ngr):
                        st_group(g)
                        pv_group(g - 1)
                    pv_group(ngr - 1)

                    nc.vector.tensor_copy(o_f[:, h, :], ot[:])

                # batched Z gather + per-head normalize
                nc.sync.dma_start(zpl[:], o_f[64:65, :, :])
                for h in range(HL):
                    zbc = smpool.tile([64, 512], f32, tag="zbc")
                    nc.gpsimd.partition_broadcast(zbc[:], zpl[0:1, h, :])
                    nc.vector.reciprocal_approx_fast(zbc[:], zbc[:])
                    if h == 0:
                        nc.vector.tensor_mul(o_pk[0:64, :], o_f[0:64, 0, :], zbc[:])
                    elif h == 1:
                        nc.vector.tensor_mul(o_m1[:], o_f[0:64, 1, :], zbc[:])
                        nc.sync.dma_start(o_pk[64:128, :], o_m1[:])
                    else:
                        nc.vector.tensor_mul(o_j2[:], o_f[0:64, 2, :], zbc[:])

                return o_pk, o_j2

            from collections import deque
            emit_proj(0)
            bpools_cm = [
                tc.tile_pool(name="ptpool", bufs=8),
                tc.tile_pool(name="smpool", bufs=2),
                tc.tile_pool(name="ytpool", bufs=2),
            ]
            ptpool, smpool, ytpool = [cm.__enter__() for cm in bpools_cm]
            # process order: biggest blocks early (rich filler overlap),
            # smallest block last (shortest exp-chase tail).
            order = [0, 2, 3, 1]
            proj_needed = {0: [1, 2], 2: [3], 3: [], 1: []}
            prev_yt = []
            for j in order:
                fillers = deque(prev_yt)
                for t in proj_needed[j]:
                    fillers.extend(proj_units(t))
                o_pk, o_j2 = emit_attention(j, ptpool, smpool, fillers)
                while fillers:
                    fillers.popleft()()
                prev_yt = yt_units(j, o_pk, o_j2, ytpool)
            for u in prev_yt:
                u()

            for cm in reversed(bpools_cm):
                cm.__exit__(None, None, None)
            xpool_cm.__exit__(None, None, None)

    nc.compile()
    _CACHED_NC = nc
    return nc


def _make_in_maps(x, Wq, Wk, Wv, Wo):
    bf16 = ml_dtypes.bfloat16
    x = np.asarray(x, np.float32)
    Wq = np.asarray(Wq, np.float32)
    Wk = np.asarray(Wk, np.float32)
    Wv = np.asarray(Wv, np.float32)
    Wo = np.asarray(Wo, np.float32)
    in_maps = []
    for c in range(8):
        b, hb = divmod(c, 4)
        s = slice(hb * HD, (hb + 1) * HD)
        wq_s = Wq[s].T  # (768, 192)
        wk_s = Wk[s].T
        # m0 = [Q0|Q1], m1 = [K0|K1], m2 = [Q2|K2]
        wqk = np.concatenate(
            [wq_s[:, 0:128], wk_s[:, 0:128],
             wq_s[:, 128:HD], wk_s[:, 128:HD]], axis=1)  # (768, 384)
        in_maps.append({
            "xt": np.ascontiguousarray(x[b].T).astype(bf16),
            "wqk": np.ascontiguousarray(wqk).astype(bf16),
            "wv": np.ascontiguousarray(Wv[s].T).astype(bf16),
            "wo": np.ascontiguousarray(Wo[:, s].T).astype(bf16),
        })
    return in_maps


def _gather(results, bo):
    out = np.zeros((B, N, C), np.float32)
    for c in range(8):
        out[c // 4] += results[c]["yt"].astype(np.float32).T
    out += np.asarray(bo, np.float32)[None, None, :]
    return out


def kernel(x, Wq, Wk, Wv, Wo, bo):
    from concourse.bass_utils import run_bass_kernel_spmd
    nc = _build_nc()
    in_maps = _make_in_maps(x, Wq, Wk, Wv, Wo)
    try:
        res = run_bass_kernel_spmd(nc, in_maps, core_ids=list(range(8)))
    except ModuleNotFoundError:
        # BASS_TRACE set but this axon deployment lacks the NTFF hook module
        import os
        os.environ["BASS_NEVER_TRACE"] = "1"
        res = run_bass_kernel_spmd(nc, in_maps, core_ids=list(range(8)))
    return _gather(res.results, bo)



# revision 6
# speedup vs baseline: 1.2496x; 1.2496x over previous
"""Causal multi-head attention (B=2, N=2048, C=768, H=12, D=64) on 8 trn2 cores.

Sharding: 8 cores = 2 batches x 4 head-blocks (3 heads each). Each core
computes q/k/v projections for its 3 heads, causal flash-attention, and a
partial output projection (its 192 columns of Wo). Host sums the 4 partials
per batch (the "all-reduce") and adds the bias during the gather.

v3: PV matmuls flipped to [query_part, d_free] layout (lhsT = pt chunks,
rhs = v[128, 65]): PE rows drop from 512/kb to 65/kb (causally exact, per
128-query block). The softmax Z lands in the free dim (column 64), so the
normalize is a per-partition broadcast multiply on DVE - no zpl DMA gather,
no gpsimd partition_broadcast, no o_m1 SBUF-SBUF DMA. Normalized o
[q, 3, 64] bf16 is transposed back to [hd, q] via PE-transpose pairs that
directly produce the o_pk ([h0;h1] at partitions 0-127) and o_j2 packing
the output projection wants. S^T matmuls get per-kb causal clipping (the
exp stays group-clipped; the 128-wide stale-psum band it may read is never
consumed downstream). Load order front-loads x(t0)+wqk(m0) across queues.
"""

import numpy as np
import ml_dtypes

B, N, C = 2, 2048, 768
H, D = 12, 64
HL = 3            # heads per core
HD = HL * D       # 192
KC = C // 128     # 6 contraction chunks
NT = N // 512     # 4 query blocks
SCALE = D ** -0.5  # 0.125

_CACHED_NC = None


def _build_nc():
    global _CACHED_NC
    if _CACHED_NC is not None:
        return _CACHED_NC
    import concourse.mybir as mybir
    import concourse.tile as tile
    from concourse import bacc

    f32 = mybir.dt.float32
    bf16 = mybir.dt.bfloat16
    Exp = mybir.ActivationFunctionType.Exp
    is_ge = mybir.AluOpType.is_ge
    is_eq = mybir.AluOpType.is_equal

    nc = bacc.Bacc("TRN2", target_bir_lowering=False, debug=False,
                   enable_asserts=False, num_devices=8)
    xt_d = nc.dram_tensor("xt", [C, N], bf16, kind="ExternalInput").ap()
    wqk_d = nc.dram_tensor("wqk", [C, 384], bf16, kind="ExternalInput").ap()
    wv_d = nc.dram_tensor("wv", [C, 192], bf16, kind="ExternalInput").ap()
    wo_d = nc.dram_tensor("wo", [HD, C], bf16, kind="ExternalInput").ap()
    yt_d = nc.dram_tensor("yt", [C, N], bf16, kind="ExternalOutput").ap()

    # per-head (m-chunk, base-partition) in qkt_sb. K2 lives in k2t_sb.
    QLOC = [(0, 0), (0, 64), (2, 0)]
    KLOC = [(1, 0), (1, 64), None]

    with tile.TileContext(nc) as tc:
        with (
            tc.tile_pool(name="wpool", bufs=1) as wpool,
            tc.tile_pool(name="qkpool", bufs=1) as qkpool,
            tc.tile_pool(name="vpool", bufs=1) as vpool,
            tc.tile_pool(name="opool", bufs=2) as opool,
            tc.tile_pool(name="ppsum", bufs=2, space="PSUM") as ppsum,
            tc.tile_pool(name="stpsum", bufs=2, space="PSUM") as stpsum,
            tc.tile_pool(name="pvpsum", bufs=2, space="PSUM") as pvpsum,
        ):
            wqk_sb = wpool.tile([128, KC, 384], bf16)
            wv_sb = wpool.tile([128, KC, 192], bf16)
            wopk_sb = wpool.tile([128, C], bf16)
            wo2_sb = wpool.tile([64, C], bf16)
            qkt_sb = qkpool.tile([128, 3, N], bf16)
            k2t_sb = qkpool.tile([64, N], bf16)
            v_sb = vpool.tile([128, HL * 16, 65], bf16)
            ones_sb = wpool.tile([128, 1], bf16)
            ident_sb = wpool.tile([128, 128], bf16)

            xpool_cm = tc.tile_pool(name="xpool", bufs=1)
            xpool = xpool_cm.__enter__()
            xt_sb = xpool.tile([128, KC, N], bf16)

            # ---- loads: x(t0) + wqk(m0) first so PE can start asap ----
            xt_r = xt_d.rearrange("(c p) n -> p c n", p=128)
            wqk_r = wqk_d.rearrange("(c p) m -> p c m", p=128)
            wv_r = wv_d.rearrange("(c p) m -> p c m", p=128)
            nc.sync.dma_start(xt_sb[:, 0:2, 0:512], xt_r[:, 0:2, 0:512])
            nc.scalar.dma_start(xt_sb[:, 2:4, 0:512], xt_r[:, 2:4, 0:512])
            nc.sync.dma_start(wqk_sb[:, :, 0:128], wqk_r[:, :, 0:128])
            nc.scalar.dma_start(xt_sb[:, 4:6, 0:512], xt_r[:, 4:6, 0:512])
            nc.gpsimd.dma_start(wv_sb[:], wv_r[:])
            nc.scalar.dma_start(wqk_sb[:, :, 128:256], wqk_r[:, :, 128:256])
            nc.sync.dma_start(wqk_sb[:, :, 256:384], wqk_r[:, :, 256:384])
            nc.vector.memset(ones_sb[:], 1.0)
            nc.vector.tensor_copy(v_sb[:, :, 64:65],
                                  ones_sb[:].to_broadcast([128, HL * 16, 1]))
            nc.gpsimd.memset(ident_sb[:], 1.0)
            nc.gpsimd.affine_select(ident_sb[:], ident_sb[:],
                                    pattern=[[1, 128]], compare_op=is_eq,
                                    fill=0.0, base=0, channel_multiplier=-1)
            engs = [nc.sync, nc.scalar]
            for t in range(1, NT):
                for cp in range(3):
                    cs = slice(cp * 2, cp * 2 + 2)
                    sl = slice(t * 512, (t + 1) * 512)
                    engs[(t + cp) % 2].dma_start(xt_sb[:, cs, sl],
                                                 xt_r[:, cs, sl])
            nc.scalar.dma_start(wopk_sb[:], wo_d[0:128, :])
            nc.gpsimd.dma_start(wo2_sb[:], wo_d[128:HD, :])

            # ---- projections, as independent psum-group units (PE fillers) ----
            def proj_units(t):
                ts_ = slice(t * 512, (t + 1) * 512)
                units = []

                def qk_unit(m):
                    def emit():
                        ps = ppsum.tile([128, 512], f32, tag="proj")
                        for c in range(KC):
                            nc.tensor.matmul(ps[:], wqk_sb[:, c, m * 128:(m + 1) * 128],
                                             xt_sb[:, c, ts_],
                                             start=(c == 0), stop=(c == KC - 1))
                        nc.vector.tensor_copy(qkt_sb[:, m, ts_], ps[:])
                        if m == 2:
                            nc.sync.dma_start(k2t_sb[:, ts_],
                                              qkt_sb[64:128, 2, ts_])
                    return emit

                def v_unit(q):
                    def emit():
                        tok = t * 512 + q * 128
                        i = t * 4 + q
                        ps = ppsum.tile([128, 512], f32, tag="proj")
                        for c in range(KC):
                            nc.tensor.matmul(ps[:, 0:192], xt_sb[:, c, tok:tok + 128],
                                             wv_sb[:, c, :],
                                             start=(c == 0), stop=(c == KC - 1))
                        vsrc = ps[:, 0:HD].rearrange("p (h d) -> p h d", h=HL)
                        nc.vector.tensor_copy(v_sb[:, i::16, 0:64], vsrc)
                    return emit

                for m in (0, 1):
                    units.append(qk_unit(m))
                for q in range(4):
                    units.append(v_unit(q))
                units.append(qk_unit(2))
                return units

            def emit_proj(t):
                for u in proj_units(t):
                    u()

            def yt_units(j, o_pk, o_j2, ytpool, last):
                qs = slice(j * 512, (j + 1) * 512)
                yt_r = yt_d.rearrange("(c p) n -> p c n", p=128)
                yt_sb = ytpool.tile([128, KC, 512], bf16, tag="yt")
                units = []

                def ob_unit(ob):
                    def emit():
                        ps = ppsum.tile([128, 512], f32, tag="proj")
                        nc.tensor.matmul(ps[:], wopk_sb[:, ob * 128:(ob + 1) * 128],
                                         o_pk[:], start=True, stop=False)
                        nc.tensor.matmul(ps[:], wo2_sb[:, ob * 128:(ob + 1) * 128],
                                         o_j2[:], start=False, stop=True)
                        if last and ob % 2 == 0:
                            nc.scalar.copy(yt_sb[:, ob, :], ps[:])
                        else:
                            nc.vector.tensor_copy(yt_sb[:, ob, :], ps[:])
                        if ob == KC - 1:
                            nc.sync.dma_start(yt_r[:, :, qs], yt_sb[:])
                    return emit

                return [ob_unit(ob) for ob in range(KC)]

            def emit_attention(j, ptpool, fillers):
                """S^T + exp (per head) and contiguous per-(h, qb) PV bursts.
                A psum bank cannot hold two accumulation chains with
                interleaved matmuls, so each (h, qb) region accumulates in
                one contiguous start->stop run; completed regions survive
                later regions\' starts in the same bank. Head h\'s bursts run
                after head h+1\'s S phase so the exp chase is hidden."""
                qs0 = j * 512
                ngr = 2 * (j + 1)
                pv01 = pvpsum.tile([128, 2, HL, 65], f32, tag="pv")
                pv23 = pvpsum.tile([128, 2, HL, 65], f32, tag="pv")
                pvt = [pv01, pv23]
                nslots = HL * (ngr + 4)
                cadence = max(1, round(nslots / (len(fillers) + 1)))
                slot = [0]

                def maybe_fill():
                    slot[0] += 1
                    if fillers and slot[0] % cadence == 0:
                        fillers.popleft()()

                pts_h = [[] for _ in range(HL)]

                def st_group(h, g):
                    qm, qp = QLOC[h]
                    glo = 128 * (2 * g - 4 * j) if 2 * g >= 4 * j else 0
                    st = stpsum.tile([128, 2, 512], f32, tag="st")
                    pt = ptpool.tile([128, 2, 512], bf16, tag="pt")
                    pts_h[h].append(pt)
                    for li in range(2):
                        kb = 2 * g + li
                        gl = 128 * (kb - 4 * j) if kb >= 4 * j else 0
                        if h < 2:
                            km, kp = KLOC[h]
                            lhsk = qkt_sb[kp:kp + 64, km,
                                          kb * 128:(kb + 1) * 128]
                        else:
                            lhsk = k2t_sb[:, kb * 128:(kb + 1) * 128]
                        nc.tensor.matmul(
                            st[:, li, gl:512], lhsk,
                            qkt_sb[qp:qp + 64, qm, qs0 + gl:qs0 + 512],
                            start=True, stop=True)
                    nc.scalar.activation(pt[:, :, glo:512], st[:, :, glo:512],
                                         Exp, scale=SCALE)
                    for li in range(2):
                        kb = 2 * g + li
                        if kb >= 4 * j:  # diagonal: mask the 128-wide band
                            di = kb - 4 * j
                            blk = pt[:, li, 128 * di:128 * (di + 1)]
                            nc.gpsimd.affine_select(
                                blk, blk, pattern=[[1, 128]], compare_op=is_ge,
                                fill=0.0, base=0, channel_multiplier=-1)

                def burst(h, qb):
                    last_kb = 4 * j + qb
                    for kb in range(last_kb + 1):
                        pt = pts_h[h][kb // 2]
                        nc.tensor.matmul(
                            pvt[qb // 2][:, qb % 2, h, :],
                            pt[:, kb % 2, qb * 128:(qb + 1) * 128],
                            v_sb[:, h * 16 + kb, :],
                            start=(kb == 0), stop=(kb == last_kb))

                for g in range(ngr):
                    st_group(0, g)
                    maybe_fill()
                for g in range(ngr):
                    st_group(1, g)
                    maybe_fill()
                for qb in range(4):
                    burst(0, qb)
                    maybe_fill()
                for g in range(ngr):
                    st_group(2, g)
                    maybe_fill()
                for qb in range(4):
                    burst(1, qb)
                    maybe_fill()
                for qb in range(4):
                    burst(2, qb)
                    maybe_fill()
                return pvt

            def finish_block(pvt, smpool):
                """normalize (per-partition 1/Z) + transpose back to [hd, q]."""
                zr = smpool.tile([128, 2, 2, HL, 1], f32, tag="zr")
                o_nrm = smpool.tile([128, 4, HL, 64], bf16, tag="on")
                o_pk = opool.tile([128, 512], bf16, tag="opk")
                o_j2 = opool.tile([64, 512], bf16, tag="oj2")
                tp = ppsum.tile([128, 1024], bf16, tag="proj")
                for pair in range(2):
                    ps = pvt[pair]
                    nc.vector.reciprocal_approx_fast(
                        zr[:, pair, :, :, :].rearrange("p a h 1 -> p (a h) 1"),
                        ps[:, :, :, 64:65].rearrange("p a h 1 -> p (a h) 1"))
                    nc.vector.tensor_mul(
                        o_nrm[:, pair * 2:pair * 2 + 2, :, :], ps[:, :, :, 0:64],
                        zr[:, pair, :, :, :].to_broadcast([128, 2, HL, 64]))
                for qb in range(4):
                    qsl = slice(qb * 128, (qb + 1) * 128)
                    nc.tensor.transpose(tp[:, qsl], o_nrm[:, qb, 0:2, :],
                                        ident_sb[:])
                    qsl2 = slice(512 + qb * 128, 512 + (qb + 1) * 128)
                    nc.tensor.transpose(tp[0:64, qsl2], o_nrm[:, qb, 2, :],
                                        ident_sb[:])
                nc.vector.tensor_copy(o_pk[:], tp[:, 0:512])
                nc.vector.tensor_copy(o_j2[:], tp[0:64, 512:1024])
                return o_pk, o_j2

            from collections import deque
            emit_proj(0)
            bpools_cm = [
                tc.tile_pool(name="ptpool", bufs=16),
                tc.tile_pool(name="smpool", bufs=2),
                tc.tile_pool(name="ytpool", bufs=2),
            ]
            ptpool, smpool, ytpool = [cm.__enter__() for cm in bpools_cm]
            # process order: biggest blocks early (rich filler overlap),
            # smallest block last (shortest exp-chase tail).
            order = [0, 2, 3, 1]
            proj_needed = {0: [1, 2], 2: [3], 3: [], 1: []}
            prev_yt = []
            for oi, j in enumerate(order):
                fillers = deque(prev_yt)
                for t in proj_needed[j]:
                    fillers.extend(proj_units(t))
                pvt = emit_attention(j, ptpool, fillers)
                while fillers:
                    fillers.popleft()()
                o_pk, o_j2 = finish_block(pvt, smpool)
                prev_yt = yt_units(j, o_pk, o_j2, ytpool, last=(oi == 3))
            for u in prev_yt:
                u()

            for cm in reversed(bpools_cm):
                cm.__exit__(None, None, None)
            xpool_cm.__exit__(None, None, None)

    nc.compile()
    _CACHED_NC = nc
    return nc


def _make_in_maps(x, Wq, Wk, Wv, Wo):
    bf16 = ml_dtypes.bfloat16
    x = np.asarray(x, np.float32)
    Wq = np.asarray(Wq, np.float32)
    Wk = np.asarray(Wk, np.float32)
    Wv = np.asarray(Wv, np.float32)
    Wo = np.asarray(Wo, np.float32)
    in_maps = []
    for c in range(8):
        b, hb = divmod(c, 4)
        s = slice(hb * HD, (hb + 1) * HD)
        wq_s = Wq[s].T  # (768, 192)
        wk_s = Wk[s].T
        # m0 = [Q0|Q1], m1 = [K0|K1], m2 = [Q2|K2]
        wqk = np.concatenate(
            [wq_s[:, 0:128], wk_s[:, 0:128],
             wq_s[:, 128:HD], wk_s[:, 128:HD]], axis=1)  # (768, 384)
        in_maps.append({
            "xt": np.ascontiguousarray(x[b].T).astype(bf16),
            "wqk": np.ascontiguousarray(wqk).astype(bf16),
            "wv": np.ascontiguousarray(Wv[s].T).astype(bf16),
            "wo": np.ascontiguousarray(Wo[:, s].T).astype(bf16),
        })
    return in_maps


def _gather(results, bo):
    out = np.zeros((B, N, C), np.float32)
    for c in range(8):
        out[c // 4] += results[c]["yt"].astype(np.float32).T
    out += np.asarray(bo, np.float32)[None, None, :]
    return out


def kernel(x, Wq, Wk, Wv, Wo, bo):
    from concourse.bass_utils import run_bass_kernel_spmd
    nc = _build_nc()
    in_maps = _make_in_maps(x, Wq, Wk, Wv, Wo)
    try:
        res = run_bass_kernel_spmd(nc, in_maps, core_ids=list(range(8)))
    except ModuleNotFoundError:
        # BASS_TRACE set but this axon deployment lacks the NTFF hook module
        import os
        os.environ["BASS_NEVER_TRACE"] = "1"
        res = run_bass_kernel_spmd(nc, in_maps, core_ids=list(range(8)))
    return _gather(res.results, bo)


# revision 8
# speedup vs baseline: 1.3035x; 1.0431x over previous
"""Causal multi-head attention (B=2, N=2048, C=768, H=12, D=64) on 8 trn2 cores.

Sharding: 8 cores = 2 batches x 4 head-blocks (3 heads each). Each core
computes q/k/v projections for its 3 heads, causal flash-attention, and a
partial output projection (its 192 columns of Wo). Host sums the 4 partials
per batch (the "all-reduce") and adds the bias during the gather.

v3: PV matmuls flipped to [query_part, d_free] layout (lhsT = pt chunks,
rhs = v[128, 65]): PE rows drop from 512/kb to 65/kb (causally exact, per
128-query block). The softmax Z lands in the free dim (column 64), so the
normalize is a per-partition broadcast multiply on DVE - no zpl DMA gather,
no gpsimd partition_broadcast, no o_m1 SBUF-SBUF DMA. Normalized o
[q, 3, 64] bf16 is transposed back to [hd, q] via PE-transpose pairs that
directly produce the o_pk ([h0;h1] at partitions 0-127) and o_j2 packing
the output projection wants. S^T matmuls get per-kb causal clipping (the
exp stays group-clipped; the 128-wide stale-psum band it may read is never
consumed downstream). Load order front-loads x(t0)+wqk(m0) across queues.
"""

import numpy as np
import ml_dtypes

B, N, C = 2, 2048, 768
H, D = 12, 64
HL = 3            # heads per core
HD = HL * D       # 192
KC = C // 128     # 6 contraction chunks
NT = N // 512     # 4 query blocks
SCALE = D ** -0.5  # 0.125

_CACHED_NC = None


def _build_nc():
    global _CACHED_NC
    if _CACHED_NC is not None:
        return _CACHED_NC
    import concourse.mybir as mybir
    import concourse.tile as tile
    from concourse import bacc

    f32 = mybir.dt.float32
    bf16 = mybir.dt.bfloat16
    Exp = mybir.ActivationFunctionType.Exp
    is_ge = mybir.AluOpType.is_ge
    is_eq = mybir.AluOpType.is_equal

    nc = bacc.Bacc("TRN2", target_bir_lowering=False, debug=False,
                   enable_asserts=False, num_devices=8)
    xt_d = nc.dram_tensor("xt", [C, N], bf16, kind="ExternalInput").ap()
    wqk_d = nc.dram_tensor("wqk", [C, 384], bf16, kind="ExternalInput").ap()
    wv_d = nc.dram_tensor("wv", [C, 192], bf16, kind="ExternalInput").ap()
    wo_d = nc.dram_tensor("wo", [HD, C], bf16, kind="ExternalInput").ap()
    yt_d = nc.dram_tensor("yt", [C, N], bf16, kind="ExternalOutput").ap()

    # per-head (m-chunk, base-partition) in qkt_sb. K2 lives in k2t_sb.
    QLOC = [(0, 0), (0, 64), (2, 0)]
    KLOC = [(1, 0), (1, 64), None]

    with tile.TileContext(nc) as tc:
        with (
            tc.tile_pool(name="wpool", bufs=1) as wpool,
            tc.tile_pool(name="qkpool", bufs=1) as qkpool,
            tc.tile_pool(name="vpool", bufs=1) as vpool,
            tc.tile_pool(name="opool", bufs=2) as opool,
            tc.tile_pool(name="ppsum", bufs=2, space="PSUM") as ppsum,
            tc.tile_pool(name="stpsum", bufs=2, space="PSUM") as stpsum,
            tc.tile_pool(name="pvpsum", bufs=2, space="PSUM") as pvpsum,
        ):
            wqk_sb = wpool.tile([128, KC, 384], bf16)
            wv_sb = wpool.tile([128, KC, 192], bf16)
            wopk_sb = wpool.tile([128, C], bf16)
            wo2_sb = wpool.tile([64, C], bf16)
            qkt_sb = qkpool.tile([128, 3, N], bf16)
            k2t_sb = qkpool.tile([64, N], bf16)
            v_sb = vpool.tile([128, HL * 16, 65], bf16)
            ones_sb = wpool.tile([128, 1], bf16)
            ident_sb = wpool.tile([128, 128], bf16)

            xpool_cm = tc.tile_pool(name="xpool", bufs=1)
            xpool = xpool_cm.__enter__()
            xt_sb = xpool.tile([128, KC, N], bf16)

            # ---- loads: x(t0) + wqk(m0) first so PE can start asap ----
            xt_r = xt_d.rearrange("(c p) n -> p c n", p=128)
            wqk_r = wqk_d.rearrange("(c p) m -> p c m", p=128)
            wv_r = wv_d.rearrange("(c p) m -> p c m", p=128)
            # PE p-state warmup: dummy matmuls on never-written garbage so
            # the ramp (low->mid->full over 3us) completes before real work.
            dummy_sb = wpool.tile([128, 512], bf16)
            nc.vector.memset(dummy_sb[:], 1.0)
            wups = ppsum.tile([128, 512], f32, tag="proj")
            for _ in range(6):
                nc.tensor.matmul(wups[:], dummy_sb[:, 0:128], dummy_sb[:],
                                 start=True, stop=True)
            nc.sync.dma_start(xt_sb[:, 0:3, 0:512], xt_r[:, 0:3, 0:512])
            nc.scalar.dma_start(wqk_sb[:, :, 0:128], wqk_r[:, :, 0:128])
            nc.sync.dma_start(xt_sb[:, 3:6, 0:512], xt_r[:, 3:6, 0:512])
            nc.scalar.dma_start(wqk_sb[:, :, 128:384], wqk_r[:, :, 128:384])
            nc.gpsimd.dma_start(wv_sb[:], wv_r[:])
            nc.vector.memset(ones_sb[:], 1.0)
            nc.vector.tensor_copy(v_sb[:, :, 64:65],
                                  ones_sb[:].to_broadcast([128, HL * 16, 1]))
            nc.gpsimd.memset(ident_sb[:], 1.0)
            nc.gpsimd.affine_select(ident_sb[:], ident_sb[:],
                                    pattern=[[1, 128]], compare_op=is_eq,
                                    fill=0.0, base=0, channel_multiplier=-1)
            engs = [nc.sync, nc.scalar]
            for t in range(1, NT):
                sl = slice(t * 512, (t + 1) * 512)
                engs[t % 2].dma_start(xt_sb[:, :, sl], xt_r[:, :, sl])
            nc.gpsimd.dma_start(wopk_sb[:], wo_d[0:128, :])
            nc.gpsimd.dma_start(wo2_sb[:], wo_d[128:HD, :])

            # ---- projections, as independent psum-group units (PE fillers) ----
            def proj_units(t):
                ts_ = slice(t * 512, (t + 1) * 512)
                units = []

                def qk_unit(m):
                    def emit():
                        ps = ppsum.tile([128, 512], f32, tag="proj")
                        for c in range(KC):
                            nc.tensor.matmul(ps[:], wqk_sb[:, c, m * 128:(m + 1) * 128],
                                             xt_sb[:, c, ts_],
                                             start=(c == 0), stop=(c == KC - 1))
                        nc.vector.tensor_copy(qkt_sb[:, m, ts_], ps[:])
                        if m == 2:
                            nc.gpsimd.dma_start(k2t_sb[:, ts_],
                                                qkt_sb[64:128, 2, ts_])
                    return emit

                def v_unit(q):
                    def emit():
                        tok = t * 512 + q * 128
                        i = t * 4 + q
                        ps = ppsum.tile([128, 512], f32, tag="proj")
                        for c in range(KC):
                            nc.tensor.matmul(ps[:, 0:192], xt_sb[:, c, tok:tok + 128],
                                             wv_sb[:, c, :],
                                             start=(c == 0), stop=(c == KC - 1))
                        vsrc = ps[:, 0:HD].rearrange("p (h d) -> p h d", h=HL)
                        nc.vector.tensor_copy(v_sb[:, i::16, 0:64], vsrc)
                    return emit

                for m in (0, 1):
                    units.append(qk_unit(m))
                for q in range(4):
                    units.append(v_unit(q))
                units.append(qk_unit(2))
                return units

            def emit_proj(t):
                for u in proj_units(t):
                    u()

            def yt_units(j, o_pk, o_j2, ytpool, last):
                qs = slice(j * 512, (j + 1) * 512)
                yt_r = yt_d.rearrange("(c p) n -> p c n", p=128)
                yt_sb = ytpool.tile([128, KC, 512], bf16, tag="yt")
                units = []

                def ob_unit(ob):
                    def emit():
                        ps = ppsum.tile([128, 512], f32, tag="proj")
                        nc.tensor.matmul(ps[:], wopk_sb[:, ob * 128:(ob + 1) * 128],
                                         o_pk[:], start=True, stop=False)
                        nc.tensor.matmul(ps[:], wo2_sb[:, ob * 128:(ob + 1) * 128],
                                         o_j2[:], start=False, stop=True)
                        if last and ob % 2 == 0:
                            nc.scalar.copy(yt_sb[:, ob, :], ps[:])
                        else:
                            nc.vector.tensor_copy(yt_sb[:, ob, :], ps[:])
                        if ob == KC - 1:
                            eng = nc.sync if last else nc.gpsimd
                            eng.dma_start(yt_r[:, :, qs], yt_sb[:])
                    return emit

                return [ob_unit(ob) for ob in range(KC)]

            def emit_attention(j, ptpool, fillers, mid_units=()):
                """S^T + exp (per head) and contiguous per-(h, qb) PV bursts.
                A psum bank cannot hold two accumulation chains with
                interleaved matmuls, so each (h, qb) region accumulates in
                one contiguous start->stop run; completed regions survive
                later regions\' starts in the same bank. Head h\'s bursts run
                after head h+1\'s S phase so the exp chase is hidden."""
                qs0 = j * 512
                ngr = 2 * (j + 1)
                pv01 = pvpsum.tile([128, 2, HL, 65], f32, tag="pv")
                pv23 = pvpsum.tile([128, 2, HL, 65], f32, tag="pv")
                pvt = [pv01, pv23]
                nslots = HL * (ngr + 4)
                cadence = max(1, round(nslots / (len(fillers) + 1)))
                slot = [0]

                def maybe_fill():
                    slot[0] += 1
                    if fillers and slot[0] % cadence == 0:
                        fillers.popleft()()

                pts_h = [[] for _ in range(HL)]

                def st_group(h, g):
                    qm, qp = QLOC[h]
                    glo = 128 * (2 * g - 4 * j) if 2 * g >= 4 * j else 0
                    st = stpsum.tile([128, 2, 512], f32, tag="st")
                    pt = ptpool.tile([128, 2, 512], bf16, tag="pt")
                    pts_h[h].append(pt)
                    for li in range(2):
                        kb = 2 * g + li
                        gl = 128 * (kb - 4 * j) if kb >= 4 * j else 0
                        if h < 2:
                            km, kp = KLOC[h]
                            lhsk = qkt_sb[kp:kp + 64, km,
                                          kb * 128:(kb + 1) * 128]
                        else:
                            lhsk = k2t_sb[:, kb * 128:(kb + 1) * 128]
                        nc.tensor.matmul(
                            st[:, li, gl:512], lhsk,
                            qkt_sb[qp:qp + 64, qm, qs0 + gl:qs0 + 512],
                            start=True, stop=True)
                    nc.scalar.activation(pt[:, :, glo:512], st[:, :, glo:512],
                                         Exp, scale=SCALE)
                    for li in range(2):
                        kb = 2 * g + li
                        if kb >= 4 * j:  # diagonal: mask the 128-wide band
                            di = kb - 4 * j
                            blk = pt[:, li, 128 * di:128 * (di + 1)]
                            nc.gpsimd.affine_select(
                                blk, blk, pattern=[[1, 128]], compare_op=is_ge,
                                fill=0.0, base=0, channel_multiplier=-1)

                def burst(h, qb):
                    last_kb = 4 * j + qb
                    for kb in range(last_kb + 1):
                        pt = pts_h[h][kb // 2]
                        nc.tensor.matmul(
                            pvt[qb // 2][:, qb % 2, h, :],
                            pt[:, kb % 2, qb * 128:(qb + 1) * 128],
                            v_sb[:, h * 16 + kb, :],
                            start=(kb == 0), stop=(kb == last_kb))

                for g in range(ngr):
                    st_group(0, g)
                    maybe_fill()
                for g in range(ngr):
                    st_group(1, g)
                    maybe_fill()
                for u in mid_units:
                    u()
                for qb in range(4):
                    burst(0, qb)
                    maybe_fill()
                for g in range(ngr):
                    st_group(2, g)
                    maybe_fill()
                for qb in range(4):
                    burst(1, qb)
                    maybe_fill()
                for qb in range(4):
                    burst(2, qb)
                    maybe_fill()
                return pvt

            def finish_block(pvt, smpool):
                """normalize (per-partition 1/Z) + transpose back to [hd, q]."""
                zr = smpool.tile([128, 2, 2, HL, 1], f32, tag="zr")
                o_nrm = smpool.tile([128, 4, HL, 64], bf16, tag="on")
                o_pk = opool.tile([128, 512], bf16, tag="opk")
                o_j2 = opool.tile([64, 512], bf16, tag="oj2")
                tp = ppsum.tile([128, 1024], bf16, tag="proj")
                for pair in range(2):
                    ps = pvt[pair]
                    nc.vector.reciprocal_approx_fast(
                        zr[:, pair, :, :, :].rearrange("p a h 1 -> p (a h) 1"),
                        ps[:, :, :, 64:65].rearrange("p a h 1 -> p (a h) 1"))
                    nc.vector.tensor_mul(
                        o_nrm[:, pair * 2:pair * 2 + 2, :, :], ps[:, :, :, 0:64],
                        zr[:, pair, :, :, :].to_broadcast([128, 2, HL, 64]))
                for qb in range(4):
                    qsl = slice(qb * 128, (qb + 1) * 128)
                    nc.tensor.transpose(tp[:, qsl], o_nrm[:, qb, 0:2, :],
                                        ident_sb[:])
                    qsl2 = slice(512 + qb * 128, 512 + (qb + 1) * 128)
                    nc.tensor.transpose(tp[0:64, qsl2], o_nrm[:, qb, 2, :],
                                        ident_sb[:])
                nc.vector.tensor_copy(o_pk[:], tp[:, 0:512])
                nc.vector.tensor_copy(o_j2[:], tp[0:64, 512:1024])
                return o_pk, o_j2

            from collections import deque
            emit_proj(0)
            bpools_cm = [
                tc.tile_pool(name="ptpool", bufs=16),
                tc.tile_pool(name="smpool", bufs=2),
                tc.tile_pool(name="ytpool", bufs=2),
            ]
            ptpool, smpool, ytpool = [cm.__enter__() for cm in bpools_cm]
            # process order: biggest blocks early (rich filler overlap),
            # smallest block last (shortest exp-chase tail).
            order = [0, 2, 3, 1]
            # t3's qk0/qk1 ride block 2's filler stream; its qk2+v units are
            # emitted mid-block-3 (after the st(1) phase, before any burst
            # that reads v(t3)) to feed PE during block 3's exp-bound phases.
            pu3 = proj_units(3)
            proj_fill = {0: proj_units(1) + proj_units(2),
                         2: [pu3[0], pu3[1]], 3: [], 1: []}
            mid = {0: (), 2: (), 3: (pu3[6], pu3[2], pu3[3], pu3[4], pu3[5]),
                   1: ()}
            prev_yt = []
            for oi, j in enumerate(order):
                fillers = deque(prev_yt)
                fillers.extend(proj_fill[j])
                pvt = emit_attention(j, ptpool, fillers, mid_units=mid[j])
                while fillers:
                    fillers.popleft()()
                o_pk, o_j2 = finish_block(pvt, smpool)
                prev_yt = yt_units(j, o_pk, o_j2, ytpool, last=(oi == 3))
            for u in prev_yt:
                u()

            for cm in reversed(bpools_cm):
                cm.__exit__(None, None, None)
            xpool_cm.__exit__(None, None, None)

    nc.compile()
    _CACHED_NC = nc
    return nc


def _make_in_maps(x, Wq, Wk, Wv, Wo):
    bf16 = ml_dtypes.bfloat16
    x = np.asarray(x, np.float32)
    Wq = np.asarray(Wq, np.float32)
    Wk = np.asarray(Wk, np.float32)
    Wv = np.asarray(Wv, np.float32)
    Wo = np.asarray(Wo, np.float32)
    in_maps = []
    for c in range(8):
        b, hb = divmod(c, 4)
        s = slice(hb * HD, (hb + 1) * HD)
        wq_s = Wq[s].T  # (768, 192)
        wk_s = Wk[s].T
        # m0 = [Q0|Q1], m1 = [K0|K1], m2 = [Q2|K2]
        wqk = np.concatenate(
            [wq_s[:, 0:128], wk_s[:, 0:128],
             wq_s[:, 128:HD], wk_s[:, 128:HD]], axis=1)  # (768, 384)
        in_maps.append({
            "xt": np.ascontiguousarray(x[b].T).astype(bf16),
            "wqk": np.ascontiguousarray(wqk).astype(bf16),
            "wv": np.ascontiguousarray(Wv[s].T).astype(bf16),
            "wo": np.ascontiguousarray(Wo[:, s].T).astype(bf16),
        })
    return in_maps


def _gather(results, bo):
    out = np.zeros((B, N, C), np.float32)
    for c in range(8):
        out[c // 4] += results[c]["yt"].astype(np.float32).T
    out += np.asarray(bo, np.float32)[None, None, :]
    return out


def kernel(x, Wq, Wk, Wv, Wo, bo):
    from concourse.bass_utils import run_bass_kernel_spmd
    nc = _build_nc()
    in_maps = _make_in_maps(x, Wq, Wk, Wv, Wo)
    try:
        res = run_bass_kernel_spmd(nc, in_maps, core_ids=list(range(8)))
    except ModuleNotFoundError:
        # BASS_TRACE set but this axon deployment lacks the NTFF hook module
        import os
        os.environ["BASS_NEVER_TRACE"] = "1"
        res = run_bass_kernel_spmd(nc, in_maps, core_ids=list(range(8)))
    return _gather(res.results, bo)


# revision 12
# speedup vs baseline: 1.3098x; 1.0049x over previous
"""Causal multi-head attention (B=2, N=2048, C=768, H=12, D=64) on 8 trn2 cores.

Sharding: 8 cores = 2 batches x 4 head-blocks (3 heads each). Each core
computes q/k/v projections for its 3 heads, causal flash-attention, and a
partial output projection (its 192 columns of Wo). Host sums the 4 partials
per batch (the "all-reduce") and adds the bias during the gather.

v3: PV matmuls flipped to [query_part, d_free] layout (lhsT = pt chunks,
rhs = v[128, 65]): PE rows drop from 512/kb to 65/kb (causally exact, per
128-query block). The softmax Z lands in the free dim (column 64), so the
normalize is a per-partition broadcast multiply on DVE - no zpl DMA gather,
no gpsimd partition_broadcast, no o_m1 SBUF-SBUF DMA. Normalized o
[q, 3, 64] bf16 is transposed back to [hd, q] via PE-transpose pairs that
directly produce the o_pk ([h0;h1] at partitions 0-127) and o_j2 packing
the output projection wants. S^T matmuls get per-kb causal clipping (the
exp stays group-clipped; the 128-wide stale-psum band it may read is never
consumed downstream). Load order front-loads x(t0)+wqk(m0) across queues.
"""

import numpy as np
import ml_dtypes

B, N, C = 2, 2048, 768
H, D = 12, 64
HL = 3            # heads per core
HD = HL * D       # 192
KC = C // 128     # 6 contraction chunks
NT = N // 512     # 4 query blocks
SCALE = D ** -0.5  # 0.125

_CACHED_NC = None


def _build_nc():
    global _CACHED_NC
    if _CACHED_NC is not None:
        return _CACHED_NC
    import concourse.mybir as mybir
    import concourse.tile as tile
    from concourse import bacc

    f32 = mybir.dt.float32
    bf16 = mybir.dt.bfloat16
    Exp = mybir.ActivationFunctionType.Exp
    is_ge = mybir.AluOpType.is_ge
    is_eq = mybir.AluOpType.is_equal

    nc = bacc.Bacc("TRN2", target_bir_lowering=False, debug=False,
                   enable_asserts=False, num_devices=8)
    xt_d = nc.dram_tensor("xt", [C, N], bf16, kind="ExternalInput").ap()
    wqk_d = nc.dram_tensor("wqk", [3, 128, KC, 128], bf16,
                           kind="ExternalInput").ap()
    wv_d = nc.dram_tensor("wv", [128, KC, 192], bf16,
                          kind="ExternalInput").ap()
    wo_d = nc.dram_tensor("wo", [HD, C], bf16, kind="ExternalInput").ap()
    yt_d = nc.dram_tensor("yt", [C, N], bf16, kind="ExternalOutput").ap()

    # per-head (m-chunk, base-partition) in qkt_sb. K2 lives in k2t_sb.
    QLOC = [(0, 0), (0, 64), (2, 0)]
    KLOC = [(1, 0), (1, 64), None]

    with tile.TileContext(nc) as tc:
        with (
            tc.tile_pool(name="wpool", bufs=1) as wpool,
            tc.tile_pool(name="qkpool", bufs=1) as qkpool,
            tc.tile_pool(name="vpool", bufs=1) as vpool,
            tc.tile_pool(name="opool", bufs=2) as opool,
            tc.tile_pool(name="ppsum", bufs=2, space="PSUM") as ppsum,
            tc.tile_pool(name="stpsum", bufs=2, space="PSUM") as stpsum,
            tc.tile_pool(name="pvpsum", bufs=2, space="PSUM") as pvpsum,
        ):
            wqk_sb = wpool.tile([128, KC, 384], bf16)
            wv_sb = wpool.tile([128, KC, 192], bf16)
            wopk_sb = wpool.tile([128, C], bf16)
            wo2_sb = wpool.tile([64, C], bf16)
            qkt_sb = qkpool.tile([128, 3, N], bf16)
            k2t_sb = qkpool.tile([64, N], bf16)
            v_sb = vpool.tile([128, HL * 16, 65], bf16)
            ones_sb = wpool.tile([128, 1], bf16)
            ident_sb = wpool.tile([128, 128], bf16)

            xpool_cm = tc.tile_pool(name="xpool", bufs=1)
            xpool = xpool_cm.__enter__()
            xt_sb = xpool.tile([128, KC, N], bf16)

            # ---- loads: wqk(m0) + x(t0) first, on the fast queues ----
            xt_r = xt_d.rearrange("(c p) n -> p c n", p=128)
            # PE p-state warmup: dummy matmuls on a memset tile so the ramp
            # (low->mid->full over 3us) completes before real work arrives.
            dummy_sb = wpool.tile([128, 512], bf16)
            nc.vector.memset(dummy_sb[:], 1.0)
            wups = ppsum.tile([128, 512], f32, tag="proj")
            for _ in range(6):
                nc.tensor.matmul(wups[:], dummy_sb[:, 0:128], dummy_sb[:],
                                 start=True, stop=True)
            nc.sync.dma_start(wqk_sb[:, :, 0:128], wqk_d[0])
            nc.gpsimd.dma_start(xt_sb[:, 0:3, 0:512], xt_r[:, 0:3, 0:512])
            nc.sync.dma_start(xt_sb[:, 3:6, 0:512], xt_r[:, 3:6, 0:512])
            nc.scalar.dma_start(wqk_sb[:, :, 128:256], wqk_d[1])
            nc.scalar.dma_start(wqk_sb[:, :, 256:384], wqk_d[2])
            nc.gpsimd.dma_start(wv_sb[:], wv_d[:])
            nc.vector.memset(ones_sb[:], 1.0)
            nc.vector.tensor_copy(v_sb[:, :, 64:65],
                                  ones_sb[:].to_broadcast([128, HL * 16, 1]))
            nc.gpsimd.memset(ident_sb[:], 1.0)
            nc.gpsimd.affine_select(ident_sb[:], ident_sb[:],
                                    pattern=[[1, 128]], compare_op=is_eq,
                                    fill=0.0, base=0, channel_multiplier=-1)
            engs = [nc.sync, nc.scalar]
            for t in range(1, NT):
                sl = slice(t * 512, (t + 1) * 512)
                engs[t % 2].dma_start(xt_sb[:, :, sl], xt_r[:, :, sl])
            nc.gpsimd.dma_start(wopk_sb[:], wo_d[0:128, :])
            nc.gpsimd.dma_start(wo2_sb[:], wo_d[128:HD, :])

            # ---- projections, as independent psum-group units (PE fillers) ----
            def proj_units(t):
                ts_ = slice(t * 512, (t + 1) * 512)
                units = []

                def qk_unit(m):
                    def emit():
                        ps = ppsum.tile([128, 512], f32, tag="proj")
                        for c in range(KC):
                            nc.tensor.matmul(ps[:], wqk_sb[:, c, m * 128:(m + 1) * 128],
                                             xt_sb[:, c, ts_],
                                             start=(c == 0), stop=(c == KC - 1))
                        nc.vector.tensor_copy(qkt_sb[:, m, ts_], ps[:])
                        if m == 2:
                            nc.gpsimd.dma_start(k2t_sb[:, ts_],
                                                qkt_sb[64:128, 2, ts_])
                    return emit

                def v_unit(q):
                    def emit():
                        tok = t * 512 + q * 128
                        i = t * 4 + q
                        ps = ppsum.tile([128, 512], f32, tag="proj")
                        for c in range(KC):
                            nc.tensor.matmul(ps[:, 0:192], xt_sb[:, c, tok:tok + 128],
                                             wv_sb[:, c, :],
                                             start=(c == 0), stop=(c == KC - 1))
                        vsrc = ps[:, 0:HD].rearrange("p (h d) -> p h d", h=HL)
                        nc.vector.tensor_copy(v_sb[:, i::16, 0:64], vsrc)
                    return emit

                for m in (0, 1):
                    units.append(qk_unit(m))
                for q in range(4):
                    units.append(v_unit(q))
                units.append(qk_unit(2))
                return units

            def emit_proj(t):
                for u in proj_units(t):
                    u()

            def yt_units(j, o_pk, o_j2, ytpool, last):
                qs = slice(j * 512, (j + 1) * 512)
                yt_r = yt_d.rearrange("(c p) n -> p c n", p=128)
                yt_sb = ytpool.tile([128, KC, 512], bf16, tag="yt")
                units = []

                def ob_unit(ob):
                    def emit():
                        ps = ppsum.tile([128, 512], f32, tag="proj")
                        nc.tensor.matmul(ps[:], wopk_sb[:, ob * 128:(ob + 1) * 128],
                                         o_pk[:], start=True, stop=False)
                        nc.tensor.matmul(ps[:], wo2_sb[:, ob * 128:(ob + 1) * 128],
                                         o_j2[:], start=False, stop=True)
                        if last and ob % 2 == 0:
                            nc.scalar.copy(yt_sb[:, ob, :], ps[:])
                        else:
                            nc.vector.tensor_copy(yt_sb[:, ob, :], ps[:])
                        if last and ob == 2:
                            nc.sync.dma_start(yt_r[:, 0:3, qs], yt_sb[:, 0:3, :])
                        elif last and ob == KC - 1:
                            nc.sync.dma_start(yt_r[:, 3:6, qs], yt_sb[:, 3:6, :])
                        elif ob == KC - 1:
                            nc.gpsimd.dma_start(yt_r[:, :, qs], yt_sb[:])
                    return emit

                return [ob_unit(ob) for ob in range(KC)]

            def emit_attention(j, ptpool, smpool, fillers, mid_units=()):
                """S^T + exp (per head) and contiguous per-(h, qb) PV bursts.
                A psum bank cannot hold two accumulation chains with
                interleaved matmuls, so each (h, qb) region accumulates in
                one contiguous start->stop run; completed regions survive
                later regions\' starts in the same bank. Head h\'s bursts run
                after head h+1\'s S phase so the exp chase is hidden."""
                qs0 = j * 512
                ngr = 2 * (j + 1)
                pv01 = pvpsum.tile([128, 2, HL, 65], f32, tag="pv")
                pv23 = pvpsum.tile([128, 2, HL, 65], f32, tag="pv")
                pvt = [pv01, pv23]
                nslots = HL * (ngr + 4)
                cadence = max(1, round(nslots / (len(fillers) + 1)))
                slot = [0]

                def maybe_fill():
                    slot[0] += 1
                    if fillers and slot[0] % cadence == 0:
                        fillers.popleft()()

                pts_h = [[] for _ in range(HL)]

                def st_group(h, g):
                    qm, qp = QLOC[h]
                    glo = 128 * (2 * g - 4 * j) if 2 * g >= 4 * j else 0
                    st = stpsum.tile([128, 2, 512], f32, tag="st")
                    pt = ptpool.tile([128, 2, 512], bf16, tag="pt")
                    pts_h[h].append(pt)
                    for li in range(2):
                        kb = 2 * g + li
                        gl = 128 * (kb - 4 * j) if kb >= 4 * j else 0
                        if h < 2:
                            km, kp = KLOC[h]
                            lhsk = qkt_sb[kp:kp + 64, km,
                                          kb * 128:(kb + 1) * 128]
                        else:
                            lhsk = k2t_sb[:, kb * 128:(kb + 1) * 128]
                        nc.tensor.matmul(
                            st[:, li, gl:512], lhsk,
                            qkt_sb[qp:qp + 64, qm, qs0 + gl:qs0 + 512],
                            start=True, stop=True)
                    nc.scalar.activation(pt[:, :, glo:512], st[:, :, glo:512],
                                         Exp, scale=SCALE)
                    for li in range(2):
                        kb = 2 * g + li
                        if kb >= 4 * j:  # diagonal: mask the 128-wide band
                            di = kb - 4 * j
                            blk = pt[:, li, 128 * di:128 * (di + 1)]
                            nc.gpsimd.affine_select(
                                blk, blk, pattern=[[1, 128]], compare_op=is_ge,
                                fill=0.0, base=0, channel_multiplier=-1)

                def burst(h, qb):
                    last_kb = 4 * j + qb
                    for kb in range(last_kb + 1):
                        pt = pts_h[h][kb // 2]
                        nc.tensor.matmul(
                            pvt[qb // 2][:, qb % 2, h, :],
                            pt[:, kb % 2, qb * 128:(qb + 1) * 128],
                            v_sb[:, h * 16 + kb, :],
                            start=(kb == 0), stop=(kb == last_kb))

                for g in range(ngr):
                    st_group(0, g)
                    maybe_fill()
                for g in range(ngr):
                    st_group(1, g)
                    maybe_fill()
                for u in mid_units:
                    u()
                for qb in range(4):
                    burst(0, qb)
                    maybe_fill()
                for g in range(ngr):
                    st_group(2, g)
                    maybe_fill()
                for qb in range(4):
                    burst(1, qb)
                    maybe_fill()
                # last head: finish each qb-pair as soon as its regions stop,
                # so normalize/transpose/copies overlap the remaining bursts
                zr = smpool.tile([128, 2, 2, HL, 1], f32, tag="zr")
                o_nrm = smpool.tile([128, 4, HL, 64], bf16, tag="on")
                o_pk = opool.tile([128, 512], bf16, tag="opk")
                o_j2 = opool.tile([64, 512], bf16, tag="oj2")
                tp = ppsum.tile([128, 1024], bf16, tag="proj")

                def finish_pair(pair):
                    ps = pvt[pair]
                    nc.vector.reciprocal_approx_fast(
                        zr[:, pair, :, :, :].rearrange("p a h 1 -> p (a h) 1"),
                        ps[:, :, :, 64:65].rearrange("p a h 1 -> p (a h) 1"))
                    nc.vector.tensor_mul(
                        o_nrm[:, pair * 2:pair * 2 + 2, :, :], ps[:, :, :, 0:64],
                        zr[:, pair, :, :, :].to_broadcast([128, 2, HL, 64]))
                    for qb in (pair * 2, pair * 2 + 1):
                        qsl = slice(qb * 128, (qb + 1) * 128)
                        nc.tensor.transpose(tp[:, qsl], o_nrm[:, qb, 0:2, :],
                                            ident_sb[:])
                        qsl2 = slice(512 + qb * 128, 512 + (qb + 1) * 128)
                        nc.tensor.transpose(tp[0:64, qsl2], o_nrm[:, qb, 2, :],
                                            ident_sb[:])
                    hsl = slice(pair * 256, (pair + 1) * 256)
                    nc.vector.tensor_copy(o_pk[:, hsl], tp[:, hsl])
                    nc.vector.tensor_copy(
                        o_j2[:, hsl], tp[0:64, 512 + pair * 256:768 + pair * 256])

                for qb in range(4):
                    burst(2, qb)
                    maybe_fill()
                    if qb == 1:
                        finish_pair(0)
                    elif qb == 3:
                        finish_pair(1)
                return o_pk, o_j2

            from collections import deque
            emit_proj(0)
            bpools_cm = [
                tc.tile_pool(name="ptpool", bufs=16),
                tc.tile_pool(name="smpool", bufs=2),
                tc.tile_pool(name="ytpool", bufs=2),
            ]
            ptpool, smpool, ytpool = [cm.__enter__() for cm in bpools_cm]
            # process order: biggest blocks early (rich filler overlap),
            # smallest block last (shortest exp-chase tail).
            order = [0, 2, 3, 1]
            # t3's qk0/qk1 ride block 2's filler stream; its qk2+v units are
            # emitted mid-block-3 (after the st(1) phase, before any burst
            # that reads v(t3)) to feed PE during block 3's exp-bound phases.
            pu3 = proj_units(3)
            proj_fill = {0: proj_units(1) + proj_units(2),
                         2: [pu3[0], pu3[1]], 3: [], 1: []}
            mid = {0: (), 2: (), 3: (pu3[6], pu3[2], pu3[3], pu3[4], pu3[5]),
                   1: ()}
            prev_yt = []
            for oi, j in enumerate(order):
                fillers = deque(prev_yt)
                fillers.extend(proj_fill[j])
                o_pk, o_j2 = emit_attention(j, ptpool, smpool, fillers,
                                              mid_units=mid[j])
                while fillers:
                    fillers.popleft()()
                prev_yt = yt_units(j, o_pk, o_j2, ytpool, last=(oi == 3))
            for u in prev_yt:
                u()

            for cm in reversed(bpools_cm):
                cm.__exit__(None, None, None)
            xpool_cm.__exit__(None, None, None)

    nc.compile()
    _CACHED_NC = nc
    return nc


def _make_in_maps(x, Wq, Wk, Wv, Wo):
    bf16 = ml_dtypes.bfloat16
    x = np.asarray(x, np.float32)
    Wq = np.asarray(Wq, np.float32)
    Wk = np.asarray(Wk, np.float32)
    Wv = np.asarray(Wv, np.float32)
    Wo = np.asarray(Wo, np.float32)
    in_maps = []
    for c in range(8):
        b, hb = divmod(c, 4)
        s = slice(hb * HD, (hb + 1) * HD)
        wq_s = Wq[s].T  # (768, 192)
        wk_s = Wk[s].T
        # m0 = [Q0|Q1], m1 = [K0|K1], m2 = [Q2|K2]; each m-block stored
        # partition-major [128p, 6c, 128m] so every partition's DMA payload
        # is one contiguous 1536B run (128 descriptors, not 768).
        blocks = [np.concatenate([wq_s[:, 0:128], wk_s[:, 0:128]], axis=1),
                  np.concatenate([wq_s[:, 128:HD], wk_s[:, 128:HD]], axis=1)]
        wqk = np.stack([
            np.concatenate([blocks[0][:, 0:128]], axis=1),
            np.concatenate([blocks[0][:, 128:256]], axis=1),
            blocks[1],
        ]).reshape(3, KC, 128, 128).transpose(0, 2, 1, 3)  # (3, p, c, m)
        wv_pm = Wv[s].T.reshape(KC, 128, 192).transpose(1, 0, 2)  # (p, c, m)
        in_maps.append({
            "xt": np.ascontiguousarray(x[b].T).astype(bf16),
            "wqk": np.ascontiguousarray(wqk).astype(bf16),
            "wv": np.ascontiguousarray(wv_pm).astype(bf16),
            "wo": np.ascontiguousarray(Wo[:, s].T).astype(bf16),
        })
    return in_maps


def _gather(results, bo):
    out = np.zeros((B, N, C), np.float32)
    for c in range(8):
        out[c // 4] += results[c]["yt"].astype(np.float32).T
    out += np.asarray(bo, np.float32)[None, None, :]
    return out


def kernel(x, Wq, Wk, Wv, Wo, bo):
    from concourse.bass_utils import run_bass_kernel_spmd
    nc = _build_nc()
    in_maps = _make_in_maps(x, Wq, Wk, Wv, Wo)
    try:
        res = run_bass_kernel_spmd(nc, in_maps, core_ids=list(range(8)))
    except ModuleNotFoundError:
        # BASS_TRACE set but this axon deployment lacks the NTFF hook module
        import os
        os.environ["BASS_NEVER_TRACE"] = "1"
        res = run_bass_kernel_spmd(nc, in_maps, core_ids=list(range(8)))
    return _gather(res.results, bo)
